# revision 1
# baseline (speedup 1.0000x reference)
"""AttentionPooling (segment softmax-pool) Trainium2 kernel.

out[s,:] = sum_n 1[idx[n]==s] * gnorm[n] * (x[n,:] @ msg_w + msg_b)
  gnorm[n] = w[n]^p * exp(gate[n]) / (denom[seg] + eps)   (max-sub skipped:
  mathematically identical after normalization, logits are O(5))

Restructured so the big matmul contracts rows via a one-hot:
  A[s,d]   = sum_n G[n,s] * x[n,d],  denom[s] = sum_n G[n,s]   (ones col)
  out[s,:] = (A[s,:] @ msg_w) / (denom+eps) + (denom/(denom+eps)) * msg_b
where G[n,s] = 1[idx[n]==s] * g[n] is built per 128-row tile with one fused
DVE tensor_scalar(is_equal, mult) against an iota row.

Sharding: index is sorted; host assigns 2048 contiguous segments per core,
16 windows x 128 segments, rows of each window padded to 66*128 = 8448.

Engine assignment (v2): PE = A-matmul + phase2; DVE = G-build, logit reduce,
small ops, phase2 copies; GPSIMD = logit multiply; ACT = exp only (ln hoisted
to one pre-pass) so its LUT never reloads.
"""

import os
import sys
import numpy as np

for _p in ("/opt/trn_rl_repo", "/root/.axon_site/_ro/trn_rl_repo"):
    if os.path.isdir(_p) and _p not in sys.path:
        sys.path.insert(0, _p)

P = 128
S = 16384
D = 128
NCORES = 8
WIN = 64                       # segments per PSUM window
NWIN = S // WIN                # 128 global windows
NWIN_CORE = NWIN // NCORES     # 16 per core
TPW = 34                       # 128-row tiles per window (padded)
GROUP = 17                     # tiles per DMA/logit super-group
GPW = TPW // GROUP             # 6 groups per window
NT = NWIN_CORE * TPW           # 1056 tiles per core
NG = NT // GROUP               # 96 groups per core
ROWS_CORE = NT * P             # 135168 padded rows per core
EPS = 1e-10

IOTA_BF16 = False              # bf16 iota regressed G-build (487 vs 266 ns)
MULT_ON_GPSIMD = False         # gpsimd streaming halves DVE via shared SBUF port
U8_MASK = True                 # host-built u8 one-hot mask kills the is_equal
G_ON_ACT_MOD = 5               # j%5 < 3 -> G-build on ACT (60%); ACT Copy+scale = g*mask
GBUILD_ON_GPSIMD = False       # gpsimd TS measured 2268ns/tile - keep on DVE
ACT_ACCUM_REDUCE = False       # 3D group reduce on DVE hits 2x mode (72ns/tile)

LAST_EXEC_NS = None
LAST_RESULTS = None

_module_cache = {}


def _build_module():
    if "nc" in _module_cache:
        return _module_cache["nc"]

    import concourse.bass as bass  # noqa: F401
    import concourse.tile as tile
    from concourse import bacc, mybir
    from concourse.masks import make_identity

    f32 = mybir.dt.float32
    bf16 = mybir.dt.bfloat16
    iota_dt = bf16 if IOTA_BF16 else f32
    AX = mybir.AxisListType
    ALU = mybir.AluOpType
    ACTF = mybir.ActivationFunctionType

    nc = bacc.Bacc(
        "TRN2",
        target_bir_lowering=False,
        debug=False,
        enable_asserts=True,
        num_devices=NCORES,
    )

    xp = nc.dram_tensor("xp", [NG * P, GROUP * (D + 1)], f32, kind="ExternalInput")
    maskg = nc.dram_tensor(
        "maskg", [NG * P, GROUP * WIN], mybir.dt.uint8, kind="ExternalInput"
    )
    wall = nc.dram_tensor("wall", [P, NT], f32, kind="ExternalInput")
    gwrep = nc.dram_tensor("gwrep", [P, GROUP * D], f32, kind="ExternalInput")
    msgw = nc.dram_tensor("msgw", [D, D], f32, kind="ExternalInput")
    msgbrep = nc.dram_tensor("msgbrep", [P, D], f32, kind="ExternalInput")
    gatebrep = nc.dram_tensor("gatebrep", [P, 1], f32, kind="ExternalInput")
    prep = nc.dram_tensor("prep", [P, 1], f32, kind="ExternalInput")
    out = nc.dram_tensor("out", [NWIN_CORE * WIN, D], f32, kind="ExternalOutput")

    with tile.TileContext(nc) as tc:
        from contextlib import ExitStack

        with ExitStack() as ctx:
            const_pool = ctx.enter_context(tc.tile_pool(name="const", bufs=1))
            xs_pool = ctx.enter_context(tc.tile_pool(name="xs", bufs=10))
            grp_pool = ctx.enter_context(tc.tile_pool(name="grp", bufs=6))
            g_pool = ctx.enter_context(tc.tile_pool(name="gm", bufs=10))
            psA_pool = ctx.enter_context(tc.tile_pool(name="psA", bufs=4, space="PSUM"))
            ps2_pool = ctx.enter_context(tc.tile_pool(name="ps2", bufs=2, space="PSUM"))
            ph2_pool = ctx.enter_context(tc.tile_pool(name="ph2", bufs=3))

            gw_t = const_pool.tile([P, GROUP * D], f32)
            nc.sync.dma_start(gw_t[:], gwrep[:, :])
            msgw_t = const_pool.tile([D, D], f32)
            nc.sync.dma_start(msgw_t[:], msgw[:, :])
            msgb_t = const_pool.tile([P, D], f32)
            nc.sync.dma_start(msgb_t[:], msgbrep[:, :])
            gateb_t = const_pool.tile([P, 1], f32)
            nc.sync.dma_start(gateb_t[:], gatebrep[:, :])
            p_t = const_pool.tile([P, 1], f32)
            nc.sync.dma_start(p_t[:], prep[:, :])
            ident = const_pool.tile([P, P], f32)
            make_identity(nc, ident[:])

            # hoisted: p*ln(w) for every tile in two ops
            w_t = const_pool.tile([P, NT], f32)
            nc.sync.dma_start(w_t[:], wall[:, :])
            plw_t = const_pool.tile([P, NT], f32)
            nc.scalar.activation(out=plw_t[:], in_=w_t[:], func=ACTF.Ln)
            nc.vector.tensor_scalar_mul(plw_t[:], plw_t[:], p_t[:, 0:1])

            gw3 = gw_t[:].rearrange("p (t d) -> p t d", d=D)

            # software pipeline: emit group g+1's logit chain before group g's
            # G-builds so exp(g+1) lands ahead of the G(g) ops in ACT's stream
            chains = {}

            def emit_chain(g):
                xs = xs_pool.tile([P, GROUP * (D + 1)], f32, tag="xs", name=f"xs{g}")
                nc.sync.dma_start(xs[:], xp[g * P : (g + 1) * P, :])
                xs3 = xs[:].rearrange("p (t d) -> p t d", d=D + 1)
                mk = xs_pool.tile(
                    [P, GROUP * WIN], mybir.dt.uint8, tag="mk", name=f"mk{g}"
                )
                nc.sync.dma_start(mk[:], maskg[g * P : (g + 1) * P, :])
                xw = grp_pool.tile([P, GROUP * D], f32, tag="xw", name=f"xw{g}")
                xw3 = xw[:].rearrange("p (t d) -> p t d", d=D)
                nc.vector.tensor_tensor(
                    out=xw3, in0=xs3[:, :, 0:D], in1=gw3, op=ALU.mult
                )
                logit = grp_pool.tile([P, GROUP], f32, tag="logit", name=f"lg{g}")
                nc.vector.reduce_sum(out=logit[:], in_=xw3, axis=AX.X)
                logit2 = grp_pool.tile([P, GROUP], f32, tag="logit2", name=f"l2{g}")
                nc.vector.tensor_add(
                    logit2[:], logit[:], plw_t[:, g * GROUP : (g + 1) * GROUP]
                )
                gex = grp_pool.tile([P, GROUP], f32, tag="gex", name=f"gx{g}")
                nc.scalar.activation(
                    out=gex[:], in_=logit2[:], func=ACTF.Exp, bias=gateb_t[:, 0:1]
                )
                chains[g] = (xs3, mk, gex)

            def emit_gmm(g, psA):
                xs3, mk, gex = chains.pop(g)
                gi = g % GPW
                for j in range(GROUP):
                    t_in_win = gi * GROUP + j
                    t_glob = g * GROUP + j
                    G = g_pool.tile([P, WIN], f32, tag="G", name=f"G{t_glob}")
                    if t_glob % 4 < 3:
                        nc.scalar.activation(
                            out=G[:],
                            in_=mk[:, j * WIN : (j + 1) * WIN],
                            func=ACTF.Copy,
                            scale=gex[:, j : j + 1],
                        )
                    else:
                        nc.vector.tensor_scalar(
                            out=G[:],
                            in0=mk[:, j * WIN : (j + 1) * WIN],
                            scalar1=gex[:, j : j + 1],
                            scalar2=None,
                            op0=ALU.mult,
                        )
                    nc.tensor.matmul(
                        out=psA[:],
                        lhsT=G[:],
                        rhs=xs3[:, j, :],
                        start=(t_in_win == 0),
                        stop=(t_in_win == TPW - 1),
                    )

            def emit_phase2(w, psA):
                sbA = ph2_pool.tile([WIN, D + 1], f32, tag="sbA", name=f"sbA{w}")
                nc.vector.tensor_copy(sbA[:], psA[:])
                deno = ph2_pool.tile([WIN, 1], f32, tag="deno", name=f"dn{w}")
                nc.vector.tensor_scalar_add(deno[:], sbA[:, D : D + 1], EPS)
                rcp = ph2_pool.tile([WIN, 1], f32, tag="rcp", name=f"rc{w}")
                nc.vector.reciprocal(out=rcp[:], in_=deno[:])
                coef = ph2_pool.tile([WIN, 1], f32, tag="coef", name=f"cf{w}")
                nc.vector.tensor_tensor(
                    out=coef[:], in0=sbA[:, D : D + 1], in1=rcp[:], op=ALU.mult
                )
                psAT = ps2_pool.tile([P, WIN], f32, tag="AT", name=f"AT{w}")
                nc.tensor.transpose(
                    out=psAT[:], in_=sbA[:, 0:D], identity=ident[:WIN, :WIN]
                )
                sbAT = ph2_pool.tile([P, WIN], f32, tag="sbAT", name=f"sT{w}")
                nc.vector.tensor_copy(sbAT[:], psAT[:])
                ps2 = ps2_pool.tile([WIN, D], f32, tag="out2", name=f"o2{w}")
                nc.tensor.matmul(
                    out=ps2[:], lhsT=sbAT[:], rhs=msgw_t[:], start=True, stop=True
                )
                outsb = ph2_pool.tile([WIN, D], f32, tag="outsb", name=f"ou{w}")
                nc.scalar.activation(
                    out=outsb[:], in_=ps2[:], func=ACTF.Copy, scale=rcp[:, 0:1]
                )
                bterm = ph2_pool.tile([WIN, D], f32, tag="bterm", name=f"bt{w}")
                nc.scalar.activation(
                    out=bterm[:], in_=msgb_t[:WIN, :], func=ACTF.Copy,
                    scale=coef[:, 0:1],
                )
                ofin = ph2_pool.tile([WIN, D], f32, tag="ofin", name=f"of{w}")
                nc.vector.tensor_add(ofin[:], outsb[:], bterm[:])
                nc.sync.dma_start(out[w * WIN : (w + 1) * WIN, :], ofin[:])

            psA_tiles = {}
            for g in range(NG):
                emit_chain(g)
                w = g // GPW
                if g % GPW == 0:
                    psA_tiles[w] = psA_pool.tile(
                        [WIN, D + 1], f32, tag="psA", name=f"psA{w}"
                    )
                emit_gmm(g, psA_tiles[w])
                if g % GPW == GPW - 1:
                    emit_phase2(w, psA_tiles.pop(w))

    nc.compile()
    _module_cache["nc"] = nc
    return nc


def _shard_inputs(x, idx, w):
    """Pad + reorder host arrays into the per-core device layouts."""
    n = idx.shape[0]
    bounds = np.searchsorted(idx, np.arange(0, S + 1, WIN)).astype(np.int64)
    counts = np.diff(bounds)
    if counts.max() > TPW * P:
        raise RuntimeError(f"window overflow: {counts.max()} > {TPW * P}")

    dest = np.arange(n, dtype=np.int64) + np.repeat(
        np.arange(NWIN, dtype=np.int64) * (TPW * P) - bounds[:-1], counts
    )

    xpad = np.zeros((NCORES * ROWS_CORE, D + 1), dtype=np.float32)
    xpad[:, D] = 1.0
    xpad[dest, 0:D] = x
    idxl = np.zeros(NCORES * ROWS_CORE, dtype=np.float32)
    idxl[dest] = (idx - np.repeat(np.arange(NWIN, dtype=np.int64) * WIN, counts)).astype(
        np.float32
    )
    wpad = np.ones(NCORES * ROWS_CORE, dtype=np.float32)
    wpad[dest] = w

    # device layout: per core, per group: [128 partitions, GROUP tiles, ...]
    xdev = (
        xpad.reshape(NCORES, NG, GROUP, P, D + 1)
        .transpose(0, 1, 3, 2, 4)
        .reshape(NCORES, NG * P, GROUP * (D + 1))
    )
    mask = np.zeros((NCORES * ROWS_CORE, WIN), dtype=np.uint8)
    mask[dest, idxl[dest].astype(np.int64)] = 1
    maskdev = (
        mask.reshape(NCORES, NG, GROUP, P, WIN)
        .transpose(0, 1, 3, 2, 4)
        .reshape(NCORES, NG * P, GROUP * WIN)
    )
    wdev = np.ascontiguousarray(wpad.reshape(NCORES, NT, P).transpose(0, 2, 1))
    return xdev, maskdev, wdev


def _ensure_ntff_hook():
    """The image's antenv package lacks axon_hooks; shim it so trace=True
    can register the ctypes NTFF hook from trn_agent_boot."""
    try:
        from antenv.axon_hooks import get_axon_ntff_profile_hook  # noqa: F401

        return True
    except ImportError:
        pass
    try:
        import types

        import antenv
        from trn_agent_boot.trn_boot import _ntff_profile_via_ctypes

        mod = types.ModuleType("antenv.axon_hooks")
        _hook = [None]
        mod.set_axon_ntff_profile_hook = lambda h: _hook.__setitem__(0, h)
        mod.get_axon_ntff_profile_hook = lambda: _hook[0]
        sys.modules["antenv.axon_hooks"] = mod
        antenv.axon_hooks = mod
        mod.set_axon_ntff_profile_hook(
            _ntff_profile_via_ctypes("/opt/axon/libaxon_pjrt.so")
        )
        return True
    except Exception as e:  # degrade to untraced run
        print(f"ntff hook install failed: {type(e).__name__}: {e}")
        return False


def kernel(x, index, weights, gate_w, gate_b, msg_w, msg_b, pow_p):
    global LAST_EXEC_NS, LAST_RESULTS

    x = np.ascontiguousarray(np.asarray(x, dtype=np.float32))
    idx = np.asarray(index).astype(np.int64).ravel()
    w = np.asarray(weights, dtype=np.float32).ravel()
    gate_w = np.asarray(gate_w, dtype=np.float32).reshape(D)
    gate_b = np.asarray(gate_b, dtype=np.float32).reshape(1)
    msg_w = np.ascontiguousarray(np.asarray(msg_w, dtype=np.float32))
    msg_b = np.asarray(msg_b, dtype=np.float32).reshape(D)
    pow_p = np.asarray(pow_p, dtype=np.float32).reshape(1)

    if not np.all(idx[1:] >= idx[:-1]):
        perm = np.argsort(idx, kind="stable")
        idx = idx[perm]
        x = x[perm]
        w = w[perm]

    xdev, maskdev, wdev = _shard_inputs(x, idx, w)

    gwrep = np.tile(gate_w[None, :], (P, GROUP)).astype(np.float32)
    msgbrep = np.tile(msg_b[None, :], (P, 1)).astype(np.float32)
    gatebrep = np.full((P, 1), gate_b[0], dtype=np.float32)
    prep = np.full((P, 1), pow_p[0], dtype=np.float32)
    nc = _build_module()
    from concourse.bass_utils import run_bass_kernel_spmd

    in_maps = []
    for c in range(NCORES):
        in_maps.append(
            {
                "xp": np.ascontiguousarray(xdev[c]),
                "maskg": np.ascontiguousarray(maskdev[c]),
                "wall": wdev[c],
                "gwrep": gwrep,
                "msgw": msg_w,
                "msgbrep": msgbrep,
                "gatebrep": gatebrep,
                "prep": prep,
            }
        )

    trace = bool(os.environ.get("KERNEL_TRACE"))
    if trace:
        trace = _ensure_ntff_hook()
    res = run_bass_kernel_spmd(
        nc, in_maps, core_ids=list(range(NCORES)), trace=trace
    )
    LAST_RESULTS = res
    LAST_EXEC_NS = res.exec_time_ns

    out = np.concatenate([res.results[c]["out"] for c in range(NCORES)], axis=0)
    return out.astype(np.float32)


def kernel_numpy(x, index, weights, gate_w, gate_b, msg_w, msg_b, pow_p):
    """Host-side mirror of the device algorithm (debug only)."""
    x = np.asarray(x, dtype=np.float32)
    idx = np.asarray(index).astype(np.int64).ravel()
    w = np.asarray(weights, dtype=np.float32).ravel()
    gate = x @ np.asarray(gate_w, dtype=np.float32).reshape(D, 1)
    gate = gate[:, 0] + np.asarray(gate_b).reshape(1)[0]
    g = np.exp(gate + np.asarray(pow_p).reshape(1)[0] * np.log(w))
    A = np.zeros((S, D), dtype=np.float64)
    den = np.zeros(S, dtype=np.float64)
    np.add.at(A, idx, g[:, None] * x)
    np.add.at(den, idx, g)
    out = (A @ np.asarray(msg_w, dtype=np.float64)) / (den[:, None] + EPS)
    out = out + (den / (den + EPS))[:, None] * np.asarray(msg_b).reshape(1, D)
    return out.astype(np.float32)



# revision 3
# speedup vs baseline: 2.1489x; 2.1489x over previous
"""AttentionPooling (segment softmax-pool) Trainium2 kernel — v2 (bf16 batched).

out[s,:] = sum_n 1[idx[n]==s] * gnorm[n] * (x[n,:] @ msg_w + msg_b)
  gnorm[n] = w[n]^p * exp(gate[n]) / (denom[seg] + eps)   (max-sub skipped:
  mathematically identical after normalization, logits are O(5))

Restructured so the big matmul contracts rows via a one-hot:
  A[s,d]   = sum_n G[n,s] * x[n,d],  denom[s] = sum_n G[n,s]   (ones col)
  out[s,:] = (A[s,:] @ msg_w) / (denom+eps) + (denom/(denom+eps)) * msg_b
where G[n,s] = 1[idx[n]==s] * g[n].

v2 changes vs v1 (647us):
- Everything bf16 on the wire and in the hot loop: x tiles, gw, mask, G.
  fp32 matmul streams at half rate on the PE; bf16 halves both DMA bytes
  and PE time, and unlocks DVE 2x_1p mode for tensor_tensor.
- Logit chain batched per GROUP=32 tiles: one TT mult (2x) + two tree adds
  (2x) + one small reduce, instead of per-group big mult + 1x reduce.
- G-build batched: one ACT broadcast-exp (stride-0 input AP) materializes
  exp(logit2) across the WIN columns, then a single DVE TT mult against the
  u8->bf16 DMA-cast mask builds all GROUP G tiles in one op. Replaces 816
  per-tile ACT copies (346ns each) per core.
- Variable tiles-per-window from the actual (sorted) index: TPW_w =
  max over cores of ceil(rows/128), so the SPMD program is identical on
  all cores but ~7% of padding work disappears.
- Phase-2 PSUM->SBUF copies moved to ACT (ScalarE is PSUM-adjacent).
- Tile layout padded to 130 cols so every per-tile bf16 block is 4B-aligned
  (260B), keeping DVE 2x mode eligible on the sliced 3D access patterns.
"""

import os
import sys
import numpy as np

for _p in ("/opt/trn_rl_repo", "/root/.axon_site/_ro/trn_rl_repo"):
    if os.path.isdir(_p) and _p not in sys.path:
        sys.path.insert(0, _p)

import ml_dtypes

BF16 = ml_dtypes.bfloat16

P = 128
S = 16384
D = 128
TD = D + 2                     # tile width: 128 feats + ones col + pad col
NCORES = 8
WIN = 64                       # segments per PSUM window
NWIN = S // WIN                # 256 global windows
NWIN_CORE = NWIN // NCORES     # 32 per core
GROUP = 32                     # tiles per DMA/logit batch
EPS = 1e-10

U8CAST = True                  # ship mask u8, SWDGE dma casts to bf16 on load

LAST_EXEC_NS = None
LAST_RESULTS = None

_module_cache = {}


def _build_module(tpw):
    """tpw: tuple of NWIN_CORE tile counts (same for all cores)."""
    key = ("v2", GROUP, tuple(tpw))
    if key in _module_cache:
        return _module_cache[key]

    import concourse.bass as bass  # noqa: F401
    import concourse.tile as tile
    from concourse import bacc, mybir
    from concourse.masks import make_identity

    f32 = mybir.dt.float32
    bf16 = mybir.dt.bfloat16
    AX = mybir.AxisListType
    ALU = mybir.AluOpType
    ACTF = mybir.ActivationFunctionType

    NT = int(sum(tpw))
    NG = (NT + GROUP - 1) // GROUP
    NTP = NG * GROUP

    # flat tile stream: (window, first, last) per real tile
    tiles = []
    for w, c in enumerate(tpw):
        for k in range(c):
            tiles.append((w, k == 0, k == c - 1))

    nc = bacc.Bacc(
        "TRN2",
        target_bir_lowering=False,
        debug=False,
        enable_asserts=True,
        num_devices=NCORES,
    )

    xp = nc.dram_tensor("xp", [NG * P, GROUP * TD], bf16, kind="ExternalInput")
    mdt = mybir.dt.uint8 if U8CAST else bf16
    maskg = nc.dram_tensor("maskg", [NG * P, GROUP * WIN], mdt, kind="ExternalInput")
    wall = nc.dram_tensor("wall", [P, NTP], f32, kind="ExternalInput")
    gwrep = nc.dram_tensor("gwrep", [P, GROUP * D], bf16, kind="ExternalInput")
    msgw = nc.dram_tensor("msgw", [D, D], f32, kind="ExternalInput")
    msgbrep = nc.dram_tensor("msgbrep", [P, D], f32, kind="ExternalInput")
    gatebrep = nc.dram_tensor("gatebrep", [P, 1], f32, kind="ExternalInput")
    prep = nc.dram_tensor("prep", [P, 1], f32, kind="ExternalInput")
    out = nc.dram_tensor("out", [NWIN_CORE * WIN, D], f32, kind="ExternalOutput")

    with tile.TileContext(nc) as tc:
        from contextlib import ExitStack

        with ExitStack() as ctx:
            const_pool = ctx.enter_context(tc.tile_pool(name="const", bufs=1))
            xs_pool = ctx.enter_context(tc.tile_pool(name="xs", bufs=4))
            mk_pool = ctx.enter_context(tc.tile_pool(name="mk", bufs=4))
            xw_pool = ctx.enter_context(tc.tile_pool(name="xw", bufs=2))
            tr_pool = ctx.enter_context(tc.tile_pool(name="tr", bufs=4))
            lg_pool = ctx.enter_context(tc.tile_pool(name="lg", bufs=6))
            g_pool = ctx.enter_context(tc.tile_pool(name="gm", bufs=3))
            psA_pool = ctx.enter_context(tc.tile_pool(name="psA", bufs=4, space="PSUM"))
            ps2_pool = ctx.enter_context(tc.tile_pool(name="ps2", bufs=2, space="PSUM"))
            ph2_pool = ctx.enter_context(tc.tile_pool(name="ph2", bufs=3))

            gw_t = const_pool.tile([P, GROUP * D], bf16)
            nc.sync.dma_start(gw_t[:], gwrep[:, :])
            msgw_t = const_pool.tile([D, D], f32)
            nc.sync.dma_start(msgw_t[:], msgw[:, :])
            msgb_t = const_pool.tile([P, D], f32)
            nc.sync.dma_start(msgb_t[:], msgbrep[:, :])
            gateb_t = const_pool.tile([P, 1], f32)
            nc.sync.dma_start(gateb_t[:], gatebrep[:, :])
            p_t = const_pool.tile([P, 1], f32)
            nc.sync.dma_start(p_t[:], prep[:, :])
            ident = const_pool.tile([P, P], f32)
            make_identity(nc, ident[:])

            # hoisted: p*ln(w) for every tile in two ops
            w_t = const_pool.tile([P, NTP], f32)
            nc.sync.dma_start(w_t[:], wall[:, :])
            plw_t = const_pool.tile([P, NTP], f32)
            nc.scalar.activation(out=plw_t[:], in_=w_t[:], func=ACTF.Ln)
            nc.vector.tensor_scalar_mul(plw_t[:], plw_t[:], p_t[:, 0:1])

            gw3 = gw_t[:].rearrange("p (t d) -> p t d", d=D)

            chains = {}

            def emit_chain(g):
                xs = xs_pool.tile([P, GROUP * TD], bf16, tag="xs", name=f"xs{g}")
                nc.sync.dma_start(xs[:], xp[g * P : (g + 1) * P, :])
                xs3 = xs[:].rearrange("p (t d) -> p t d", d=TD)
                mk = mk_pool.tile([P, GROUP * WIN], bf16, tag="mk", name=f"mk{g}")
                if U8CAST:
                    nc.gpsimd.dma_start(mk[:], maskg[g * P : (g + 1) * P, :])
                else:
                    nc.sync.dma_start(mk[:], maskg[g * P : (g + 1) * P, :])
                xw = xw_pool.tile([P, GROUP * D], bf16, tag="xw", name=f"xw{g}")
                xw3 = xw[:].rearrange("p (t d) -> p t d", d=D)
                nc.vector.tensor_tensor(
                    out=xw3, in0=xs3[:, :, 0:D], in1=gw3, op=ALU.mult
                )
                t1 = tr_pool.tile([P, GROUP * 64], bf16, tag="t1", name=f"t1{g}")
                t13 = t1[:].rearrange("p (t d) -> p t d", d=64)
                nc.vector.tensor_tensor(
                    out=t13, in0=xw3[:, :, 0:64], in1=xw3[:, :, 64:128], op=ALU.add
                )
                t2 = tr_pool.tile([P, GROUP * 32], bf16, tag="t2", name=f"t2{g}")
                t23 = t2[:].rearrange("p (t d) -> p t d", d=32)
                nc.vector.tensor_tensor(
                    out=t23, in0=t13[:, :, 0:32], in1=t13[:, :, 32:64], op=ALU.add
                )
                logit = lg_pool.tile([P, GROUP], f32, tag="lg", name=f"lg{g}")
                nc.vector.reduce_sum(out=logit[:], in_=t23, axis=AX.X)
                logit2 = lg_pool.tile([P, GROUP], f32, tag="lg2", name=f"l2{g}")
                nc.vector.tensor_add(
                    logit2[:], logit[:], plw_t[:, g * GROUP : (g + 1) * GROUP]
                )
                gexb = g_pool.tile([P, GROUP * WIN], bf16, tag="gexb", name=f"ge{g}")
                gexb3 = gexb[:].rearrange("p (t s) -> p t s", s=WIN)
                lg2b = logit2[:].unsqueeze(2).broadcast_to([P, GROUP, WIN])
                nc.scalar.activation(
                    out=gexb3, in_=lg2b, func=ACTF.Exp, bias=gateb_t[:, 0:1]
                )
                Gm = g_pool.tile([P, GROUP * WIN], bf16, tag="G", name=f"G{g}")
                nc.vector.tensor_tensor(
                    out=Gm[:], in0=mk[:], in1=gexb[:], op=ALU.mult
                )
                chains[g] = (xs3, Gm)

            def emit_phase2(w, psA):
                sbA = ph2_pool.tile([WIN, D + 1], f32, tag="sbA", name=f"sbA{w}")
                nc.scalar.copy(sbA[:], psA[:])
                deno = ph2_pool.tile([WIN, 1], f32, tag="deno", name=f"dn{w}")
                nc.vector.tensor_scalar_add(deno[:], sbA[:, D : D + 1], EPS)
                rcp = ph2_pool.tile([WIN, 1], f32, tag="rcp", name=f"rc{w}")
                nc.vector.reciprocal(out=rcp[:], in_=deno[:])
                coef = ph2_pool.tile([WIN, 1], f32, tag="coef", name=f"cf{w}")
                nc.vector.tensor_tensor(
                    out=coef[:], in0=sbA[:, D : D + 1], in1=rcp[:], op=ALU.mult
                )
                psAT = ps2_pool.tile([P, WIN], f32, tag="AT", name=f"AT{w}")
                nc.tensor.transpose(
                    out=psAT[:], in_=sbA[:, 0:D], identity=ident[:WIN, :WIN]
                )
                sbAT = ph2_pool.tile([P, WIN], f32, tag="sbAT", name=f"sT{w}")
                nc.scalar.copy(sbAT[:], psAT[:])
                ps2 = ps2_pool.tile([WIN, D], f32, tag="out2", name=f"o2{w}")
                nc.tensor.matmul(
                    out=ps2[:], lhsT=sbAT[:], rhs=msgw_t[:], start=True, stop=True
                )
                outsb = ph2_pool.tile([WIN, D], f32, tag="outsb", name=f"ou{w}")
                nc.scalar.activation(
                    out=outsb[:], in_=ps2[:], func=ACTF.Copy, scale=rcp[:, 0:1]
                )
                bterm = ph2_pool.tile([WIN, D], f32, tag="bterm", name=f"bt{w}")
                nc.scalar.activation(
                    out=bterm[:], in_=msgb_t[:WIN, :], func=ACTF.Copy,
                    scale=coef[:, 0:1],
                )
                ofin = ph2_pool.tile([WIN, D], f32, tag="ofin", name=f"of{w}")
                nc.vector.tensor_add(ofin[:], outsb[:], bterm[:])
                nc.sync.dma_start(out[w * WIN : (w + 1) * WIN, :], ofin[:])

            cur = {}
            emit_chain(0)
            for g in range(NG):
                if g + 1 < NG:
                    emit_chain(g + 1)
                xs3, Gm = chains.pop(g)
                for j in range(GROUP):
                    t = g * GROUP + j
                    if t >= NT:
                        break
                    w, first, last = tiles[t]
                    if first:
                        cur[w] = psA_pool.tile(
                            [WIN, D + 1], f32, tag="psA", name=f"psA{w}"
                        )
                    nc.tensor.matmul(
                        out=cur[w][:],
                        lhsT=Gm[:, j * WIN : (j + 1) * WIN],
                        rhs=xs3[:, j, 0 : D + 1],
                        start=first,
                        stop=last,
                    )
                    if last:
                        emit_phase2(w, cur.pop(w))

    nc.compile()
    _module_cache[key] = (nc, NT, NG)
    return _module_cache[key]


def _layout(x, idx, w):
    """Pad + reorder host arrays into the per-core device layouts.

    Returns (tpw, xdev, maskdev, wdev)."""
    n = idx.shape[0]
    bounds = np.searchsorted(idx, np.arange(0, S + 1, WIN)).astype(np.int64)
    counts = np.diff(bounds)                       # rows per global window [NWIN]
    cpw = counts.reshape(NCORES, NWIN_CORE)
    tpw = np.maximum(1, -(-cpw // P)).max(axis=0)  # tiles per window, shared
    NT = int(tpw.sum())
    NG = (NT + GROUP - 1) // GROUP
    NTP = NG * GROUP
    ROWS_CORE = NTP * P

    tile_off = np.zeros(NWIN_CORE + 1, dtype=np.int64)
    np.cumsum(tpw, out=tile_off[1:])

    wg = np.repeat(np.arange(NWIN, dtype=np.int64), counts)     # global window
    rank = np.arange(n, dtype=np.int64) - np.repeat(bounds[:-1], counts)
    core = wg // NWIN_CORE
    wl = wg % NWIN_CORE
    dest = core * ROWS_CORE + tile_off[wl] * P + rank

    xpad = np.zeros((NCORES * ROWS_CORE, TD), dtype=np.float32)
    xpad[:, D] = 1.0
    xpad[dest, 0:D] = x
    segl = (idx - wg * WIN).astype(np.int64)

    mask = np.zeros((NCORES * ROWS_CORE, WIN), dtype=np.uint8)
    mask[dest, segl] = 1
    wpad = np.ones(NCORES * ROWS_CORE, dtype=np.float32)
    wpad[dest] = w

    xdev = (
        xpad.astype(BF16)
        .reshape(NCORES, NG, GROUP, P, TD)
        .transpose(0, 1, 3, 2, 4)
        .reshape(NCORES, NG * P, GROUP * TD)
    )
    maskdev = (
        mask.reshape(NCORES, NG, GROUP, P, WIN)
        .transpose(0, 1, 3, 2, 4)
        .reshape(NCORES, NG * P, GROUP * WIN)
    )
    if not U8CAST:
        maskdev = maskdev.astype(BF16)
    wdev = np.ascontiguousarray(
        wpad.reshape(NCORES, NTP, P).transpose(0, 2, 1)
    )
    return tuple(int(t) for t in tpw), xdev, maskdev, wdev


def _ensure_ntff_hook():
    """The image's antenv package lacks axon_hooks; shim it so trace=True
    can register the ctypes NTFF hook from trn_agent_boot."""
    try:
        from antenv.axon_hooks import get_axon_ntff_profile_hook  # noqa: F401

        return True
    except ImportError:
        pass
    try:
        import types

        import antenv
        from trn_agent_boot.trn_boot import _ntff_profile_via_ctypes

        mod = types.ModuleType("antenv.axon_hooks")
        _hook = [None]
        mod.set_axon_ntff_profile_hook = lambda h: _hook.__setitem__(0, h)
        mod.get_axon_ntff_profile_hook = lambda: _hook[0]
        sys.modules["antenv.axon_hooks"] = mod
        antenv.axon_hooks = mod
        mod.set_axon_ntff_profile_hook(
            _ntff_profile_via_ctypes("/opt/axon/libaxon_pjrt.so")
        )
        return True
    except Exception as e:  # degrade to untraced run
        print(f"ntff hook install failed: {type(e).__name__}: {e}")
        return False


def kernel(x, index, weights, gate_w, gate_b, msg_w, msg_b, pow_p):
    global LAST_EXEC_NS, LAST_RESULTS

    x = np.ascontiguousarray(np.asarray(x, dtype=np.float32))
    idx = np.asarray(index).astype(np.int64).ravel()
    w = np.asarray(weights, dtype=np.float32).ravel()
    gate_w = np.asarray(gate_w, dtype=np.float32).reshape(D)
    gate_b = np.asarray(gate_b, dtype=np.float32).reshape(1)
    msg_w = np.ascontiguousarray(np.asarray(msg_w, dtype=np.float32))
    msg_b = np.asarray(msg_b, dtype=np.float32).reshape(D)
    pow_p = np.asarray(pow_p, dtype=np.float32).reshape(1)

    if not np.all(idx[1:] >= idx[:-1]):
        perm = np.argsort(idx, kind="stable")
        idx = idx[perm]
        x = x[perm]
        w = w[perm]

    tpw, xdev, maskdev, wdev = _layout(x, idx, w)

    gwrep = np.tile(gate_w[None, :], (P, GROUP)).astype(BF16)
    msgbrep = np.tile(msg_b[None, :], (P, 1)).astype(np.float32)
    gatebrep = np.full((P, 1), gate_b[0], dtype=np.float32)
    prep = np.full((P, 1), pow_p[0], dtype=np.float32)
    nc, NT, NG = _build_module(tpw)
    from concourse.bass_utils import run_bass_kernel_spmd

    in_maps = []
    for c in range(NCORES):
        in_maps.append(
            {
                "xp": np.ascontiguousarray(xdev[c]),
                "maskg": np.ascontiguousarray(maskdev[c]),
                "wall": wdev[c],
                "gwrep": gwrep,
                "msgw": msg_w,
                "msgbrep": msgbrep,
                "gatebrep": gatebrep,
                "prep": prep,
            }
        )

    trace = bool(os.environ.get("KERNEL_TRACE"))
    if trace:
        trace = _ensure_ntff_hook()
    res = run_bass_kernel_spmd(
        nc, in_maps, core_ids=list(range(NCORES)), trace=trace
    )
    LAST_RESULTS = res
    LAST_EXEC_NS = res.exec_time_ns

    out = np.concatenate([res.results[c]["out"] for c in range(NCORES)], axis=0)
    return out.astype(np.float32)


def kernel_numpy(x, index, weights, gate_w, gate_b, msg_w, msg_b, pow_p):
    """Host-side mirror of the device algorithm (debug only)."""
    x = np.asarray(x, dtype=np.float32)
    idx = np.asarray(index).astype(np.int64).ravel()
    w = np.asarray(weights, dtype=np.float32).ravel()
    xb = x.astype(BF16).astype(np.float32)
    gwb = np.asarray(gate_w, dtype=np.float32).astype(BF16).astype(np.float32)
    xw = (xb * gwb.reshape(1, D)).astype(BF16).astype(np.float32)
    t1 = (xw[:, 0:64] + xw[:, 64:128]).astype(BF16).astype(np.float32)
    t2 = (t1[:, 0:32] + t1[:, 32:64]).astype(BF16).astype(np.float32)
    gate = t2.sum(axis=1) + np.asarray(gate_b).reshape(1)[0]
    g = np.exp(gate + np.asarray(pow_p).reshape(1)[0] * np.log(w))
    g = g.astype(BF16).astype(np.float32)
    A = np.zeros((S, D), dtype=np.float64)
    den = np.zeros(S, dtype=np.float64)
    np.add.at(A, idx, g[:, None] * xb)
    np.add.at(den, idx, g)
    out = (A @ np.asarray(msg_w, dtype=np.float64)) / (den[:, None] + EPS)
    out = out + (den / (den + EPS))[:, None] * np.asarray(msg_b).reshape(1, D)
    return out.astype(np.float32)
